# revision 14
# baseline (speedup 1.0000x reference)
"""Trainium2 Bass kernel for nn_Encoder_7842610283148.

2-layer bidirectional LayerNorm-GRU encoder (V=9488, E=300, H=512, T=128,
BK=256), data-parallel over (direction x batch-quarter) on 8 NeuronCores.

Structure:
  launch 0: per core (dir d, quarter q): gi0 = LN(x @ Wih0[d].T) for all t
            (big matmul), then the 128-step recurrent chain -> y0_d.
            Backward-direction cores receive time-flipped inputs so the
            device program is identical SPMD everywhere.
  host:     reassemble y0 = [y0_f || y0_b], reshard.
  launch 1: same program shape with K_in=1024 -> y1_d, fused with
            ybar_d = (y0_d + y1_d)/2 which is the only tensor the outputs
            need.
  host:     out1 = (ybar_f + ybar_b)/2 ; out2 from ybar_b[0] and
            ybar_f[lengths].

LayerNorm per row: mean comes free from the matmul via an appended
weight column (w1 = row-mean of W), sum-of-squares via ScalarE
Square+accumulate, rsqrt via bit-trick seed + 2 Newton iterations on
VectorE (no ACT table switch - only the sigmoid set is ever loaded).
"""

import numpy as np
import ml_dtypes

import concourse.bass as bass
import concourse.mybir as mybir
import concourse.tile as tile
from concourse import bacc
from concourse.masks import make_identity
from concourse.bass_utils import run_bass_kernel_spmd

dt = mybir.dt
Alu = mybir.AluOpType
Act = mybir.ActivationFunctionType

V, E, H, G = 9488, 300, 512, 1536
T, BK = 128, 256
NCORE = 8
R = BK // 4  # 64 batch rows per core (4 quarters x 2 dirs)
EPS = 1e-5
MAGIC = 0x5F3759DF

BF = dt.float16
F32 = dt.float32
NPBF = np.float16


def _ln_stats(nc, pool, pg, rows):
    """LN stats of pg [rows, 2048] PSUM (cols 0:G = pre-acts, col G = mean).

    Returns (tb, rs, nb, nbp): tb fp16 copy of pre-acts; rs = rsqrt(var+eps);
    nb = -mean*rs; nbp = +mean*rs.
    """
    tb = pool.tile([rows, G], BF, tag="ln_tb")
    scr = pool.tile([rows, G], BF, tag="ln_scr")
    ssq = pool.tile([rows, 1], F32, tag="ln_ssq")
    m = pool.tile([rows, 1], F32, tag="ln_m")
    # evacuate (plain copy, 2x mode) + mean column copy
    nc.vector.tensor_scalar(out=tb, in0=pg[:, 0:G], scalar1=1.0, scalar2=None,
                            op0=Alu.mult)
    nc.vector.tensor_scalar(out=m, in0=pg[:, G:G + 1], scalar1=1.0,
                            scalar2=None, op0=Alu.mult)
    # sum of squares on ScalarE (Square is in the sigmoid table set)
    nc.scalar.activation(out=scr, in_=tb, func=Act.Square, accum_out=ssq)
    mm = pool.tile([rows, 1], F32, tag="ln_mm")
    veps = pool.tile([rows, 1], F32, tag="ln_veps")
    nc.vector.tensor_tensor(out=mm, in0=m, in1=m, op=Alu.mult)
    # veps = ssq/G - m^2 + eps  (eps folded via dual-scalar TS on ssq)
    nc.vector.tensor_scalar(out=veps, in0=ssq, scalar1=1.0 / G, scalar2=EPS,
                            op0=Alu.mult, op1=Alu.add)
    nc.vector.tensor_tensor(out=veps, in0=veps, in1=mm, op=Alu.subtract)
    iv = pool.tile([rows, 1], dt.int32, tag="ln_iv")
    nc.vector.tensor_scalar(out=iv, in0=veps.bitcast(dt.int32), scalar1=1,
                            scalar2=None, op0=Alu.logical_shift_right)
    nc.vector.tensor_scalar(out=iv, in0=iv, scalar1=-1, scalar2=None,
                            op0=Alu.bitwise_xor)
    nc.vector.tensor_scalar(out=iv, in0=iv, scalar1=MAGIC + 1, scalar2=None,
                            op0=Alu.add)
    y = iv.bitcast(F32)
    yy = pool.tile([rows, 1], F32, tag="ln_yy")
    w = pool.tile([rows, 1], F32, tag="ln_w")
    rs = pool.tile([rows, 1], F32, tag="ln_rs")
    for i in range(2):
        nc.vector.tensor_tensor(out=yy, in0=y, in1=y, op=Alu.mult)
        nc.vector.tensor_scalar(out=w, in0=yy, scalar1=veps, scalar2=-0.5,
                                op0=Alu.mult, op1=Alu.mult)
        nc.vector.scalar_tensor_tensor(out=rs, in0=w, scalar=1.5, in1=y,
                                       op0=Alu.add, op1=Alu.mult)
        y = rs
    nbp = pool.tile([rows, 1], F32, tag="ln_nbp")
    nb = pool.tile([rows, 1], F32, tag="ln_nb")
    nc.vector.tensor_tensor(out=nbp, in0=m, in1=rs, op=Alu.mult)
    nc.vector.tensor_scalar(out=nb, in0=nbp, scalar1=-1.0, scalar2=None,
                            op0=Alu.mult)
    return tb, rs, nb, nbp


def _ln_finish(nc, pool, m, ssq, rows):
    """rs/nb/nbp from mean m and sum-of-squares ssq (both [rows,1] fp32)."""
    mm = pool.tile([rows, 1], F32, tag="ln_mm")
    veps = pool.tile([rows, 1], F32, tag="ln_veps")
    nc.vector.tensor_tensor(out=mm, in0=m, in1=m, op=Alu.mult)
    nc.vector.tensor_scalar(out=veps, in0=ssq, scalar1=1.0 / G, scalar2=EPS,
                            op0=Alu.mult, op1=Alu.add)
    nc.vector.tensor_tensor(out=veps, in0=veps, in1=mm, op=Alu.subtract)
    iv = pool.tile([rows, 1], dt.int32, tag="ln_iv")
    nc.vector.tensor_scalar(out=iv, in0=veps.bitcast(dt.int32), scalar1=1,
                            scalar2=None, op0=Alu.logical_shift_right)
    nc.vector.tensor_scalar(out=iv, in0=iv, scalar1=-1, scalar2=None,
                            op0=Alu.bitwise_xor)
    nc.vector.tensor_scalar(out=iv, in0=iv, scalar1=MAGIC + 1, scalar2=None,
                            op0=Alu.add)
    y = iv.bitcast(F32)
    yy = pool.tile([rows, 1], F32, tag="ln_yy")
    w = pool.tile([rows, 1], F32, tag="ln_w")
    rs = pool.tile([rows, 1], F32, tag="ln_rs")
    for i in range(2):
        nc.vector.tensor_tensor(out=yy, in0=y, in1=y, op=Alu.mult)
        nc.vector.tensor_scalar(out=w, in0=yy, scalar1=veps, scalar2=-0.5,
                                op0=Alu.mult, op1=Alu.mult)
        nc.vector.scalar_tensor_tensor(out=rs, in0=w, scalar=1.5, in1=y,
                                       op0=Alu.add, op1=Alu.mult)
        y = rs
    nbp = pool.tile([rows, 1], F32, tag="ln_nbp")
    nb = pool.tile([rows, 1], F32, tag="ln_nb")
    nc.vector.tensor_tensor(out=nbp, in0=m, in1=rs, op=Alu.mult)
    nc.vector.tensor_scalar(out=nb, in0=nbp, scalar1=-1.0, scalar2=None,
                            op0=Alu.mult)
    return rs, nb, nbp


def build_launch(layer, t_steps=T):
    """Build the SPMD program for one layer. K_in = 300->384 (l0) or 1024."""
    kin_pad = 384 if layer == 0 else 1024
    KT = kin_pad // 128
    NR = t_steps * R
    GP = G + 1  # weights carry the mean column

    nc = bacc.Bacc("TRN2", target_bir_lowering=False, num_devices=NCORE)
    xt_d = nc.dram_tensor("xt", [KT, 128, NR], BF, kind="ExternalInput")
    wih_d = nc.dram_tensor("wih", [KT, 128, GP], BF, kind="ExternalInput")
    whh_d = nc.dram_tensor("whh", [4, 128, GP], BF, kind="ExternalInput")
    h0_d = nc.dram_tensor("h0", [R, H], BF, kind="ExternalInput")
    h0t_d = nc.dram_tensor("h0t", [128, 256], BF, kind="ExternalInput")
    if layer == 1:
        y0h_d = nc.dram_tensor("y0h", [t_steps, R, H], BF, kind="ExternalInput")
    yout_d = nc.dram_tensor("yout", [t_steps, R, H], BF, kind="ExternalOutput")

    with tile.TileContext(nc) as tc:
        with tc.tile_pool(name="const", bufs=1) as const, \
             tc.tile_pool(name="dram", bufs=1, space="DRAM") as dram:
            a_d = dram.tile([NR, G], BF)
            wih_sb = const.tile([128, KT, GP], BF)
            for k in range(KT):
                nc.sync.dma_start(out=wih_sb[:, k, :], in_=wih_d[k, :, :])
            whh_sb = const.tile([128, 4, GP], BF)
            for k in range(4):
                nc.sync.dma_start(out=whh_sb[:, k, :], in_=whh_d[k, :, :])
            ident = const.tile([128, 128], BF)
            make_identity(nc, ident)

            # ---------------- P1: gi = LN(x @ Wih.T) --------------------
            with tc.tile_pool(name="p1", bufs=3) as p1, \
                 tc.tile_pool(name="p1ps", bufs=2, space="PSUM") as p1ps:
                for j in range(NR // 128):
                    xt_sb = p1.tile([128, KT, 128], BF, tag="xt")
                    for k in range(KT):
                        nc.sync.dma_start(out=xt_sb[:, k, :],
                                          in_=xt_d[k, :, j * 128:(j + 1) * 128])
                    pg = p1ps.tile([128, 2048], F32, tag="pg")
                    for c in range(3):
                        cs = slice(c * 512, (c + 1) * 512)
                        for k in range(KT):
                            nc.tensor.matmul(pg[:, cs], xt_sb[:, k, :],
                                             wih_sb[:, k, cs],
                                             start=(k == 0), stop=(k == KT - 1))
                    for k in range(KT):
                        nc.tensor.matmul(pg[:, G:G + 1], xt_sb[:, k, :],
                                         wih_sb[:, k, G:G + 1],
                                         start=(k == 0), stop=(k == KT - 1))
                    tb, rs, nb, _ = _ln_stats(nc, p1, pg, 128)
                    a_sb = p1.tile([128, G], BF, tag="a_sb")
                    nc.scalar.activation(out=a_sb, in_=tb, func=Act.Identity,
                                         bias=nb, scale=rs)
                    nc.sync.dma_start(out=a_d[j * 128:(j + 1) * 128, :], in_=a_sb)

            # ---------------- P2: recurrent chain -----------------------
            with tc.tile_pool(name="p2", bufs=3) as p2, \
                 tc.tile_pool(name="hpool", bufs=2) as hpool, \
                 tc.tile_pool(name="p2ps", bufs=1, space="PSUM") as p2ps, \
                 tc.tile_pool(name="trps", bufs=2, space="PSUM") as trps:
                h = hpool.tile([R, H], BF, tag="h")
                nc.sync.dma_start(out=h, in_=h0_d[:, :])
                hT = hpool.tile([128, 256], BF, tag="hT")
                nc.sync.dma_start(out=hT, in_=h0t_d[:, :])
                for t in range(t_steps):
                    at = p2.tile([R, G], BF, tag="at")
                    nc.sync.dma_start(out=at, in_=a_d[t * R:(t + 1) * R, :])
                    pg_m = p2ps.tile([R, 1], F32, tag="pgm")
                    for k in range(4):
                        nc.tensor.matmul(pg_m, hT[:, k * 64:(k + 1) * 64],
                                         whh_sb[:, k, G:G + 1],
                                         start=(k == 0), stop=(k == 3))
                    # matmul chunk c, then immediately evacuate + square that
                    # chunk while the next chunk streams. Separate PSUM tiles
                    # per chunk so bank tracking doesn't serialize chunk c+1's
                    # matmuls behind chunk c's evacuation.
                    tb = p2.tile([R, G], BF, tag="ln_tb")
                    scr = p2.tile([R, G], BF, tag="ln_scr")
                    ssqp = p2.tile([R, 3], F32, tag="ln_ssqp")
                    m = p2.tile([R, 1], F32, tag="ln_m")
                    nc.vector.tensor_scalar(out=m, in0=pg_m,
                                            scalar1=1.0, scalar2=None,
                                            op0=Alu.mult)
                    for c in range(3):
                        cs = slice(c * 512, (c + 1) * 512)
                        pg_c = p2ps.tile([R, 512], F32, tag="pgc")
                        for k in range(4):
                            nc.tensor.matmul(pg_c, hT[:, k * 64:(k + 1) * 64],
                                             whh_sb[:, k, cs],
                                             start=(k == 0), stop=(k == 3))
                        nc.vector.tensor_scalar(out=tb[:, cs], in0=pg_c,
                                                scalar1=1.0, scalar2=None,
                                                op0=Alu.mult)
                        nc.scalar.activation(out=scr[:, cs], in_=tb[:, cs],
                                             func=Act.Square,
                                             accum_out=ssqp[:, c:c + 1])
                    ssq = p2.tile([R, 1], F32, tag="ln_ssq")
                    nc.vector.tensor_tensor(out=ssq, in0=ssqp[:, 0:1],
                                            in1=ssqp[:, 1:2], op=Alu.add)
                    nc.vector.tensor_tensor(out=ssq, in0=ssq,
                                            in1=ssqp[:, 2:3], op=Alu.add)
                    rs, nb, nbp = _ln_finish(nc, p2, m, ssq, R)
                    # keep-warm: sustained N=512 redundant matmuls fed by tail
                    # tensors so the PE accumulates real busy time
                    for a, anchor in ((0, tb), (1, scr)):
                        dm = trps.tile([64, 512], F32, tag="dmy")
                        for k in range(2):
                            nc.tensor.matmul(dm, anchor[0:64, 0:64],
                                             whh_sb[0:64, k, 0:512],
                                             start=(k == 0), stop=(k == 1))
                    dm3 = trps.tile([64, 512], F32, tag="dmy")
                    for k in range(2):
                        nc.tensor.matmul(dm3, scr[0:64, 1024:1088],
                                         whh_sb[0:64, k, 0:512],
                                         start=(k == 0), stop=(k == 1))
                    v_r = p2.tile([R, H], BF, tag="v_r")
                    nc.vector.scalar_tensor_tensor(out=v_r, in0=tb[:, 0:512],
                                                   scalar=rs, in1=at[:, 0:512],
                                                   op0=Alu.mult, op1=Alu.add)
                    r = p2.tile([R, H], BF, tag="r")
                    z = p2.tile([R, H], BF, tag="z")
                    zc = p2.tile([R, H], BF, tag="zc")
                    nc.scalar.activation(out=r, in_=v_r,
                                         func=Act.Sigmoid, bias=nb, scale=1.0)
                    v_z = p2.tile([R, H], BF, tag="v_z")
                    nc.vector.scalar_tensor_tensor(out=v_z, in0=tb[:, 512:1024],
                                                   scalar=rs, in1=at[:, 512:1024],
                                                   op0=Alu.mult, op1=Alu.add)
                    dm4 = trps.tile([64, 512], F32, tag="dmy")
                    for k in range(2):
                        nc.tensor.matmul(dm4, v_z[0:64, 0:64],
                                         whh_sb[0:64, k, 0:512],
                                         start=(k == 0), stop=(k == 1))
                    nc.scalar.activation(out=z, in_=v_z,
                                         func=Act.Sigmoid, bias=nb, scale=1.0)
                    nc.vector.tensor_scalar(out=zc, in0=z, scalar1=-1.0,
                                            scalar2=1.0, op0=Alu.mult,
                                            op1=Alu.add)
                    hn = p2.tile([R, H], BF, tag="hn")
                    nc.vector.tensor_scalar(out=hn, in0=tb[:, 1024:1536],
                                            scalar1=rs, scalar2=nb,
                                            op0=Alu.mult, op1=Alu.add)
                    w = p2.tile([R, H], BF, tag="w")
                    nc.vector.tensor_tensor(out=w, in0=r, in1=hn, op=Alu.mult)
                    e = p2.tile([R, H], BF, tag="e")
                    nc.vector.tensor_tensor(out=e, in0=w, in1=at[:, 1024:1536],
                                            op=Alu.add)
                    n = p2.tile([R, H], BF, tag="n")
                    nc.scalar.activation(out=n, in_=e, func=Act.Tanh)
                    # keep the PE HAM-warm through the elementwise tail
                    dmy = trps.tile([64, 512], F32, tag="dmy")
                    for k in range(2):
                        nc.tensor.matmul(dmy, e[0:64, 0:64],
                                         whh_sb[0:64, k, 0:512],
                                         start=(k == 0), stop=(k == 1))
                    p1t = p2.tile([R, H], BF, tag="p1t")
                    nc.vector.tensor_tensor(out=p1t, in0=z, in1=h, op=Alu.mult)
                    p2t = p2.tile([R, H], BF, tag="p2t")
                    nc.vector.tensor_tensor(out=p2t, in0=zc, in1=n, op=Alu.mult)
                    h2 = hpool.tile([R, H], BF, tag="h")
                    nc.vector.tensor_tensor(out=h2, in0=p1t, in1=p2t, op=Alu.add)
                    if layer == 0:
                        nc.sync.dma_start(out=yout_d[t, :, :], in_=h2)
                    else:
                        y0t = p2.tile([R, H], BF, tag="y0t")
                        nc.sync.dma_start(out=y0t, in_=y0h_d[t, :, :])
                        yb = p2.tile([R, H], BF, tag="yb")
                        nc.vector.scalar_tensor_tensor(out=yb, in0=h2,
                                                       scalar=0.5, in1=y0t,
                                                       op0=Alu.mult, op1=Alu.add)
                        nc.sync.dma_start(out=yout_d[t, :, :], in_=yb)
                    if t < t_steps - 1:
                        ptr = trps.tile([128, 256], BF, tag="ptr")
                        for k in range(4):
                            nc.tensor.transpose(ptr[:, k * 64:(k + 1) * 64],
                                                h2[:, k * 128:(k + 1) * 128],
                                                ident[0:64, 0:64])
                        hT2 = hpool.tile([128, 256], BF, tag="hT")
                        nc.vector.tensor_scalar(out=hT2, in0=ptr, scalar1=1.0,
                                                scalar2=None, op0=Alu.mult)
                        hT = hT2
                    h = h2
    nc.finalize()
    return nc


_launch_cache = {}


def _get_launch(layer, t_steps):
    key = (layer, t_steps)
    if key not in _launch_cache:
        _launch_cache[key] = build_launch(layer, t_steps)
    return _launch_cache[key]


def _with_mcol(wT):
    """wT: [K, G] fp32 -> [K, G+1] with col G = row-mean of W (= mean over G)."""
    k = wT.shape[0]
    out = np.zeros((k, G + 1), np.float32)
    out[:, :G] = wT
    out[:, G] = wT.mean(axis=1)
    return out


def _h0t_pack(h0v):
    return np.ascontiguousarray(
        h0v.T.reshape(4, 128, R).transpose(1, 0, 2).reshape(128, 256))


def _prep_l0(src, emb, Wih0, Whh0, gih0, ghh0, h0, t_steps):
    in_maps = []
    for c in range(NCORE):
        d, q = c // 4, c % 4
        rows = slice(q * R, (q + 1) * R)
        s = src[rows, :t_steps]
        if d == 1:
            s = s[:, ::-1]
        x = emb[s]
        x = np.transpose(x, (1, 0, 2)).reshape(t_steps * R, E)
        xt = np.zeros((384, t_steps * R), np.float32)
        xt[:E, :] = x.T
        wih = (Wih0[d] * gih0[d][:, None])
        wihp = np.zeros((384, G), np.float32)
        wihp[:E, :] = wih.T
        whh = (Whh0[d] * ghh0[d][:, None]).T
        h0v = np.broadcast_to(h0[0, d, 0], (R, H)).astype(np.float32).copy()
        in_maps.append({
            "xt": xt.reshape(3, 128, t_steps * R).astype(NPBF),
            "wih": _with_mcol(wihp).reshape(3, 128, G + 1).astype(NPBF),
            "whh": _with_mcol(whh).reshape(4, 128, G + 1).astype(NPBF),
            "h0": h0v.astype(NPBF),
            "h0t": _h0t_pack(h0v).astype(NPBF),
        })
    return in_maps


def _prep_l1(y0, Wih1, Whh1, gih1, ghh1, h0, t_steps):
    in_maps = []
    for c in range(NCORE):
        d, q = c // 4, c % 4
        rows = slice(q * R, (q + 1) * R)
        yq = y0[:, rows, :]
        if d == 1:
            yq = yq[::-1]
        x = yq.reshape(t_steps * R, 2 * H)
        wih = (Wih1[d] * gih1[d][:, None])
        whh = (Whh1[d] * ghh1[d][:, None]).T
        h0v = np.broadcast_to(h0[1, d, 0], (R, H)).astype(np.float32).copy()
        y0half = 0.5 * yq[:, :, d * H:(d + 1) * H]
        in_maps.append({
            "xt": np.ascontiguousarray(x.T).reshape(8, 128, t_steps * R).astype(NPBF),
            "wih": _with_mcol(np.ascontiguousarray(wih.T)).reshape(8, 128, G + 1).astype(NPBF),
            "whh": _with_mcol(whh).reshape(4, 128, G + 1).astype(NPBF),
            "h0": h0v.astype(NPBF),
            "h0t": _h0t_pack(h0v).astype(NPBF),
            "y0h": np.ascontiguousarray(y0half).astype(NPBF),
        })
    return in_maps


def _run(inputs, t_steps=T):
    src = np.asarray(inputs["src"])
    lengths = np.asarray(inputs["lengths"])
    emb = np.asarray(inputs["emb"], np.float32)
    h0 = np.asarray(inputs["h0"], np.float32)
    gets = lambda k: np.asarray(inputs[k], np.float32)
    Wih0, Whh0 = gets("Wih0"), gets("Whh0")
    Wih1, Whh1 = gets("Wih1"), gets("Whh1")
    gih0, ghh0 = gets("gih0"), gets("ghh0")
    gih1, ghh1 = gets("gih1"), gets("ghh1")
    for k in ("bih0", "bhh0", "bih1", "bhh1"):
        assert not np.any(np.asarray(inputs[k])), f"nonzero bias {k} unsupported"

    nc0 = _get_launch(0, t_steps)
    maps0 = _prep_l0(src, emb, Wih0, Whh0, gih0, ghh0, h0, t_steps)
    res0 = run_bass_kernel_spmd(nc0, maps0, core_ids=list(range(NCORE)))
    y0 = np.zeros((t_steps, BK, 2 * H), np.float32)
    for c in range(NCORE):
        d, q = c // 4, c % 4
        y = res0.results[c]["yout"].astype(np.float32)
        if d == 1:
            y = y[::-1]
        y0[:, q * R:(q + 1) * R, d * H:(d + 1) * H] = y

    nc1 = _get_launch(1, t_steps)
    maps1 = _prep_l1(y0, Wih1, Whh1, gih1, ghh1, h0, t_steps)
    res1 = run_bass_kernel_spmd(nc1, maps1, core_ids=list(range(NCORE)))
    ybar = np.zeros((2, t_steps, BK, H), np.float32)
    for c in range(NCORE):
        d, q = c // 4, c % 4
        y = res1.results[c]["yout"].astype(np.float32)
        if d == 1:
            y = y[::-1]
        ybar[d, :, q * R:(q + 1) * R, :] = y

    out1 = 0.5 * (ybar[0] + ybar[1])
    out1 = np.ascontiguousarray(out1.transpose(1, 0, 2))
    lens = np.clip(lengths, 0, t_steps - 1)
    out2 = 0.5 * (ybar[1, 0, :, :] + ybar[0][lens, np.arange(BK), :])
    return out1, out2


def kernel(**inputs):
    return _run(inputs, T)


# revision 16
# speedup vs baseline: 1.0519x; 1.0519x over previous
"""Trainium2 Bass kernel for nn_Encoder_7842610283148.

2-layer bidirectional LayerNorm-GRU encoder (V=9488, E=300, H=512, T=128,
BK=256), data-parallel over (direction x batch-quarter) on 8 NeuronCores.

Structure:
  launch 0: per core (dir d, quarter q): gi0 = LN(x @ Wih0[d].T) for all t
            (big matmul), then the 128-step recurrent chain -> y0_d.
            Backward-direction cores receive time-flipped inputs so the
            device program is identical SPMD everywhere.
  host:     reassemble y0 = [y0_f || y0_b], reshard.
  launch 1: same program shape with K_in=1024 -> y1_d, fused with
            ybar_d = (y0_d + y1_d)/2 which is the only tensor the outputs
            need.
  host:     out1 = (ybar_f + ybar_b)/2 ; out2 from ybar_b[0] and
            ybar_f[lengths].

LayerNorm per row: mean comes free from the matmul via an appended
weight column (w1 = row-mean of W), sum-of-squares via ScalarE
Square+accumulate, rsqrt via bit-trick seed + 2 Newton iterations on
VectorE (no ACT table switch - only the sigmoid set is ever loaded).
"""

import numpy as np
import ml_dtypes

import concourse.bass as bass
import concourse.mybir as mybir
import concourse.tile as tile
from concourse import bacc
from concourse.masks import make_identity
from concourse.bass_utils import run_bass_kernel_spmd

dt = mybir.dt
Alu = mybir.AluOpType
Act = mybir.ActivationFunctionType

V, E, H, G = 9488, 300, 512, 1536
T, BK = 128, 256
NCORE = 8
R = BK // 4  # 64 batch rows per core (4 quarters x 2 dirs)
EPS = 1e-5
MAGIC = 0x5F3759DF

BF = dt.float16
F32 = dt.float32
NPBF = np.float16


def _ln_stats(nc, pool, pg, rows):
    """LN stats of pg [rows, 2048] PSUM (cols 0:G = pre-acts, col G = mean).

    Returns (tb, rs, nb, nbp): tb fp16 copy of pre-acts; rs = rsqrt(var+eps);
    nb = -mean*rs; nbp = +mean*rs.
    """
    tb = pool.tile([rows, G], BF, tag="ln_tb")
    scr = pool.tile([rows, G], BF, tag="ln_scr")
    ssq = pool.tile([rows, 1], F32, tag="ln_ssq")
    m = pool.tile([rows, 1], F32, tag="ln_m")
    # evacuate (plain copy, 2x mode) + mean column copy
    nc.vector.tensor_scalar(out=tb, in0=pg[:, 0:G], scalar1=1.0, scalar2=None,
                            op0=Alu.mult)
    nc.vector.tensor_scalar(out=m, in0=pg[:, G:G + 1], scalar1=1.0,
                            scalar2=None, op0=Alu.mult)
    # sum of squares on ScalarE (Square is in the sigmoid table set)
    nc.scalar.activation(out=scr, in_=tb, func=Act.Square, accum_out=ssq)
    mm = pool.tile([rows, 1], F32, tag="ln_mm")
    veps = pool.tile([rows, 1], F32, tag="ln_veps")
    nc.vector.tensor_tensor(out=mm, in0=m, in1=m, op=Alu.mult)
    # veps = ssq/G - m^2 + eps  (eps folded via dual-scalar TS on ssq)
    nc.vector.tensor_scalar(out=veps, in0=ssq, scalar1=1.0 / G, scalar2=EPS,
                            op0=Alu.mult, op1=Alu.add)
    nc.vector.tensor_tensor(out=veps, in0=veps, in1=mm, op=Alu.subtract)
    iv = pool.tile([rows, 1], dt.int32, tag="ln_iv")
    nc.vector.tensor_scalar(out=iv, in0=veps.bitcast(dt.int32), scalar1=1,
                            scalar2=None, op0=Alu.logical_shift_right)
    nc.vector.tensor_scalar(out=iv, in0=iv, scalar1=-1, scalar2=None,
                            op0=Alu.bitwise_xor)
    nc.vector.tensor_scalar(out=iv, in0=iv, scalar1=MAGIC + 1, scalar2=None,
                            op0=Alu.add)
    y = iv.bitcast(F32)
    yy = pool.tile([rows, 1], F32, tag="ln_yy")
    w = pool.tile([rows, 1], F32, tag="ln_w")
    rs = pool.tile([rows, 1], F32, tag="ln_rs")
    for i in range(2):
        nc.vector.tensor_tensor(out=yy, in0=y, in1=y, op=Alu.mult)
        nc.vector.tensor_scalar(out=w, in0=yy, scalar1=veps, scalar2=-0.5,
                                op0=Alu.mult, op1=Alu.mult)
        nc.vector.scalar_tensor_tensor(out=rs, in0=w, scalar=1.5, in1=y,
                                       op0=Alu.add, op1=Alu.mult)
        y = rs
    nbp = pool.tile([rows, 1], F32, tag="ln_nbp")
    nb = pool.tile([rows, 1], F32, tag="ln_nb")
    nc.vector.tensor_tensor(out=nbp, in0=m, in1=rs, op=Alu.mult)
    nc.vector.tensor_scalar(out=nb, in0=nbp, scalar1=-1.0, scalar2=None,
                            op0=Alu.mult)
    return tb, rs, nb, nbp


def _ln_finish(nc, pool, m, ssq, rows):
    """rs/nb/nbp from mean m and sum-of-squares ssq (both [rows,1] fp32)."""
    mm = pool.tile([rows, 1], F32, tag="ln_mm")
    veps = pool.tile([rows, 1], F32, tag="ln_veps")
    nc.vector.tensor_tensor(out=mm, in0=m, in1=m, op=Alu.mult)
    nc.vector.tensor_scalar(out=veps, in0=ssq, scalar1=1.0 / G, scalar2=EPS,
                            op0=Alu.mult, op1=Alu.add)
    nc.vector.tensor_tensor(out=veps, in0=veps, in1=mm, op=Alu.subtract)
    iv = pool.tile([rows, 1], dt.int32, tag="ln_iv")
    nc.vector.tensor_scalar(out=iv, in0=veps.bitcast(dt.int32), scalar1=1,
                            scalar2=None, op0=Alu.logical_shift_right)
    nc.vector.tensor_scalar(out=iv, in0=iv, scalar1=-1, scalar2=None,
                            op0=Alu.bitwise_xor)
    nc.vector.tensor_scalar(out=iv, in0=iv, scalar1=MAGIC + 1, scalar2=None,
                            op0=Alu.add)
    y = iv.bitcast(F32)
    yy = pool.tile([rows, 1], F32, tag="ln_yy")
    w = pool.tile([rows, 1], F32, tag="ln_w")
    rs = pool.tile([rows, 1], F32, tag="ln_rs")
    for i in range(2):
        nc.vector.tensor_tensor(out=yy, in0=y, in1=y, op=Alu.mult)
        nc.vector.tensor_scalar(out=w, in0=yy, scalar1=veps, scalar2=-0.5,
                                op0=Alu.mult, op1=Alu.mult)
        nc.vector.scalar_tensor_tensor(out=rs, in0=w, scalar=1.5, in1=y,
                                       op0=Alu.add, op1=Alu.mult)
        y = rs
    nbp = pool.tile([rows, 1], F32, tag="ln_nbp")
    nb = pool.tile([rows, 1], F32, tag="ln_nb")
    nc.vector.tensor_tensor(out=nbp, in0=m, in1=rs, op=Alu.mult)
    nc.vector.tensor_scalar(out=nb, in0=nbp, scalar1=-1.0, scalar2=None,
                            op0=Alu.mult)
    return rs, nb, nbp


def build_launch(layer, t_steps=T):
    """Build the SPMD program for one layer. K_in = 300->384 (l0) or 1024."""
    kin_pad = 384 if layer == 0 else 1024
    KT = kin_pad // 128
    NR = t_steps * R
    GP = G + 1  # weights carry the mean column

    nc = bacc.Bacc("TRN2", target_bir_lowering=False, num_devices=NCORE)
    xt_d = nc.dram_tensor("xt", [KT, 128, NR], BF, kind="ExternalInput")
    wih_d = nc.dram_tensor("wih", [KT, 128, GP], BF, kind="ExternalInput")
    whh_d = nc.dram_tensor("whh", [4, 128, GP], BF, kind="ExternalInput")
    h0_d = nc.dram_tensor("h0", [R, H], BF, kind="ExternalInput")
    h0t_d = nc.dram_tensor("h0t", [128, 256], BF, kind="ExternalInput")
    if layer == 1:
        y0h_d = nc.dram_tensor("y0h", [t_steps, R, H], BF, kind="ExternalInput")
    yout_d = nc.dram_tensor("yout", [t_steps, R, H], BF, kind="ExternalOutput")

    with tile.TileContext(nc) as tc:
        with tc.tile_pool(name="const", bufs=1) as const, \
             tc.tile_pool(name="dram", bufs=1, space="DRAM") as dram:
            a_d = dram.tile([NR, G], BF)
            wih_sb = const.tile([128, KT, GP], BF)
            for k in range(KT):
                nc.sync.dma_start(out=wih_sb[:, k, :], in_=wih_d[k, :, :])
            whh_sb = const.tile([128, 4, GP], BF)
            for k in range(4):
                nc.sync.dma_start(out=whh_sb[:, k, :], in_=whh_d[k, :, :])
            ident = const.tile([128, 128], BF)
            make_identity(nc, ident)

            # ---------------- P1: gi = LN(x @ Wih.T) --------------------
            with tc.tile_pool(name="p1", bufs=3) as p1, \
                 tc.tile_pool(name="p1ps", bufs=2, space="PSUM") as p1ps:
                for j in range(NR // 128):
                    xt_sb = p1.tile([128, KT, 128], BF, tag="xt")
                    for k in range(KT):
                        nc.sync.dma_start(out=xt_sb[:, k, :],
                                          in_=xt_d[k, :, j * 128:(j + 1) * 128])
                    pg = p1ps.tile([128, 2048], F32, tag="pg")
                    for c in range(3):
                        cs = slice(c * 512, (c + 1) * 512)
                        for k in range(KT):
                            nc.tensor.matmul(pg[:, cs], xt_sb[:, k, :],
                                             wih_sb[:, k, cs],
                                             start=(k == 0), stop=(k == KT - 1))
                    for k in range(KT):
                        nc.tensor.matmul(pg[:, G:G + 1], xt_sb[:, k, :],
                                         wih_sb[:, k, G:G + 1],
                                         start=(k == 0), stop=(k == KT - 1))
                    tb, rs, nb, _ = _ln_stats(nc, p1, pg, 128)
                    a_sb = p1.tile([128, G], BF, tag="a_sb")
                    nc.scalar.activation(out=a_sb, in_=tb, func=Act.Identity,
                                         bias=nb, scale=rs)
                    nc.sync.dma_start(out=a_d[j * 128:(j + 1) * 128, :], in_=a_sb)

            # ---------------- P2: recurrent chain -----------------------
            with tc.tile_pool(name="p2", bufs=3) as p2, \
                 tc.tile_pool(name="hpool", bufs=2) as hpool, \
                 tc.tile_pool(name="p2ps", bufs=1, space="PSUM") as p2ps, \
                 tc.tile_pool(name="trps", bufs=2, space="PSUM") as trps:
                h = hpool.tile([R, H], BF, tag="h")
                nc.sync.dma_start(out=h, in_=h0_d[:, :])
                hT = hpool.tile([128, 256], BF, tag="hT")
                nc.sync.dma_start(out=hT, in_=h0t_d[:, :])
                for t in range(t_steps):
                    at = p2.tile([R, G], BF, tag="at")
                    nc.sync.dma_start(out=at, in_=a_d[t * R:(t + 1) * R, :])
                    pg_m = p2ps.tile([R, 1], F32, tag="pgm")
                    for k in range(4):
                        nc.tensor.matmul(pg_m, hT[:, k * 64:(k + 1) * 64],
                                         whh_sb[:, k, G:G + 1],
                                         start=(k == 0), stop=(k == 3))
                    # matmul chunk c, then immediately evacuate + square that
                    # chunk while the next chunk streams. Separate PSUM tiles
                    # per chunk so bank tracking doesn't serialize chunk c+1's
                    # matmuls behind chunk c's evacuation.
                    tb = p2.tile([R, G], BF, tag="ln_tb")
                    scr = p2.tile([R, G], BF, tag="ln_scr")
                    ssqp = p2.tile([R, 3], F32, tag="ln_ssqp")
                    m = p2.tile([R, 1], F32, tag="ln_m")
                    nc.vector.tensor_scalar(out=m, in0=pg_m,
                                            scalar1=1.0, scalar2=None,
                                            op0=Alu.mult)
                    for c in range(3):
                        cs = slice(c * 512, (c + 1) * 512)
                        pg_c = p2ps.tile([R, 512], F32, tag="pgc")
                        for k in range(4):
                            nc.tensor.matmul(pg_c, hT[:, k * 64:(k + 1) * 64],
                                             whh_sb[:, k, cs],
                                             start=(k == 0), stop=(k == 3))
                        nc.vector.tensor_scalar(out=tb[:, cs], in0=pg_c,
                                                scalar1=1.0, scalar2=None,
                                                op0=Alu.mult)
                        nc.scalar.activation(out=scr[:, cs], in_=tb[:, cs],
                                             func=Act.Square,
                                             accum_out=ssqp[:, c:c + 1])
                    ssq = p2.tile([R, 1], F32, tag="ln_ssq")
                    nc.vector.tensor_tensor(out=ssq, in0=ssqp[:, 0:1],
                                            in1=ssqp[:, 1:2], op=Alu.add)
                    nc.vector.tensor_tensor(out=ssq, in0=ssq,
                                            in1=ssqp[:, 2:3], op=Alu.add)
                    rs, nb, nbp = _ln_finish(nc, p2, m, ssq, R)
                    v_r = p2.tile([R, H], BF, tag="v_r")
                    nc.vector.scalar_tensor_tensor(out=v_r, in0=tb[:, 0:512],
                                                   scalar=rs, in1=at[:, 0:512],
                                                   op0=Alu.mult, op1=Alu.add)
                    r = p2.tile([R, H], BF, tag="r")
                    z = p2.tile([R, H], BF, tag="z")
                    zc = p2.tile([R, H], BF, tag="zc")
                    nc.scalar.activation(out=r, in_=v_r,
                                         func=Act.Sigmoid, bias=nb, scale=1.0)
                    v_z = p2.tile([R, H], BF, tag="v_z")
                    nc.vector.scalar_tensor_tensor(out=v_z, in0=tb[:, 512:1024],
                                                   scalar=rs, in1=at[:, 512:1024],
                                                   op0=Alu.mult, op1=Alu.add)
                    nc.scalar.activation(out=z, in_=v_z,
                                         func=Act.Sigmoid, bias=nb, scale=1.0)
                    nc.vector.tensor_scalar(out=zc, in0=z, scalar1=-1.0,
                                            scalar2=1.0, op0=Alu.mult,
                                            op1=Alu.add)
                    hn = p2.tile([R, H], BF, tag="hn")
                    nc.vector.tensor_scalar(out=hn, in0=tb[:, 1024:1536],
                                            scalar1=rs, scalar2=nb,
                                            op0=Alu.mult, op1=Alu.add)
                    w = p2.tile([R, H], BF, tag="w")
                    nc.vector.tensor_tensor(out=w, in0=r, in1=hn, op=Alu.mult)
                    e = p2.tile([R, H], BF, tag="e")
                    nc.vector.tensor_tensor(out=e, in0=w, in1=at[:, 1024:1536],
                                            op=Alu.add)
                    n = p2.tile([R, H], BF, tag="n")
                    nc.scalar.activation(out=n, in_=e, func=Act.Tanh)
                    # keep the PE HAM-warm through the elementwise tail
                    p1t = p2.tile([R, H], BF, tag="p1t")
                    nc.vector.tensor_tensor(out=p1t, in0=z, in1=h, op=Alu.mult)
                    p2t = p2.tile([R, H], BF, tag="p2t")
                    nc.vector.tensor_tensor(out=p2t, in0=zc, in1=n, op=Alu.mult)
                    h2 = hpool.tile([R, H], BF, tag="h")
                    nc.vector.tensor_tensor(out=h2, in0=p1t, in1=p2t, op=Alu.add)
                    if layer == 0:
                        nc.sync.dma_start(out=yout_d[t, :, :], in_=h2)
                    else:
                        y0t = p2.tile([R, H], BF, tag="y0t")
                        nc.sync.dma_start(out=y0t, in_=y0h_d[t, :, :])
                        yb = p2.tile([R, H], BF, tag="yb")
                        nc.vector.scalar_tensor_tensor(out=yb, in0=h2,
                                                       scalar=0.5, in1=y0t,
                                                       op0=Alu.mult, op1=Alu.add)
                        nc.sync.dma_start(out=yout_d[t, :, :], in_=yb)
                    if t < t_steps - 1:
                        ptr = trps.tile([128, 256], BF, tag="ptr")
                        for k in range(4):
                            nc.tensor.transpose(ptr[:, k * 64:(k + 1) * 64],
                                                h2[:, k * 128:(k + 1) * 128],
                                                ident[0:64, 0:64])
                        hT2 = hpool.tile([128, 256], BF, tag="hT")
                        nc.vector.tensor_scalar(out=hT2, in0=ptr, scalar1=1.0,
                                                scalar2=None, op0=Alu.mult)
                        hT = hT2
                    h = h2
    nc.finalize()
    return nc


_launch_cache = {}


def _get_launch(layer, t_steps):
    key = (layer, t_steps)
    if key not in _launch_cache:
        _launch_cache[key] = build_launch(layer, t_steps)
    return _launch_cache[key]


def _with_mcol(wT):
    """wT: [K, G] fp32 -> [K, G+1] with col G = row-mean of W (= mean over G)."""
    k = wT.shape[0]
    out = np.zeros((k, G + 1), np.float32)
    out[:, :G] = wT
    out[:, G] = wT.mean(axis=1)
    return out


def _h0t_pack(h0v):
    return np.ascontiguousarray(
        h0v.T.reshape(4, 128, R).transpose(1, 0, 2).reshape(128, 256))


def _prep_l0(src, emb, Wih0, Whh0, gih0, ghh0, h0, t_steps):
    in_maps = []
    for c in range(NCORE):
        d, q = c // 4, c % 4
        rows = slice(q * R, (q + 1) * R)
        s = src[rows, :t_steps]
        if d == 1:
            s = s[:, ::-1]
        x = emb[s]
        x = np.transpose(x, (1, 0, 2)).reshape(t_steps * R, E)
        xt = np.zeros((384, t_steps * R), np.float32)
        xt[:E, :] = x.T
        wih = (Wih0[d] * gih0[d][:, None])
        wihp = np.zeros((384, G), np.float32)
        wihp[:E, :] = wih.T
        whh = (Whh0[d] * ghh0[d][:, None]).T
        h0v = np.broadcast_to(h0[0, d, 0], (R, H)).astype(np.float32).copy()
        in_maps.append({
            "xt": xt.reshape(3, 128, t_steps * R).astype(NPBF),
            "wih": _with_mcol(wihp).reshape(3, 128, G + 1).astype(NPBF),
            "whh": _with_mcol(whh).reshape(4, 128, G + 1).astype(NPBF),
            "h0": h0v.astype(NPBF),
            "h0t": _h0t_pack(h0v).astype(NPBF),
        })
    return in_maps


def _prep_l1(y0, Wih1, Whh1, gih1, ghh1, h0, t_steps):
    in_maps = []
    for c in range(NCORE):
        d, q = c // 4, c % 4
        rows = slice(q * R, (q + 1) * R)
        yq = y0[:, rows, :]
        if d == 1:
            yq = yq[::-1]
        x = yq.reshape(t_steps * R, 2 * H)
        wih = (Wih1[d] * gih1[d][:, None])
        whh = (Whh1[d] * ghh1[d][:, None]).T
        h0v = np.broadcast_to(h0[1, d, 0], (R, H)).astype(np.float32).copy()
        y0half = 0.5 * yq[:, :, d * H:(d + 1) * H]
        in_maps.append({
            "xt": np.ascontiguousarray(x.T).reshape(8, 128, t_steps * R).astype(NPBF),
            "wih": _with_mcol(np.ascontiguousarray(wih.T)).reshape(8, 128, G + 1).astype(NPBF),
            "whh": _with_mcol(whh).reshape(4, 128, G + 1).astype(NPBF),
            "h0": h0v.astype(NPBF),
            "h0t": _h0t_pack(h0v).astype(NPBF),
            "y0h": np.ascontiguousarray(y0half).astype(NPBF),
        })
    return in_maps


def _run(inputs, t_steps=T):
    src = np.asarray(inputs["src"])
    lengths = np.asarray(inputs["lengths"])
    emb = np.asarray(inputs["emb"], np.float32)
    h0 = np.asarray(inputs["h0"], np.float32)
    gets = lambda k: np.asarray(inputs[k], np.float32)
    Wih0, Whh0 = gets("Wih0"), gets("Whh0")
    Wih1, Whh1 = gets("Wih1"), gets("Whh1")
    gih0, ghh0 = gets("gih0"), gets("ghh0")
    gih1, ghh1 = gets("gih1"), gets("ghh1")
    for k in ("bih0", "bhh0", "bih1", "bhh1"):
        assert not np.any(np.asarray(inputs[k])), f"nonzero bias {k} unsupported"

    nc0 = _get_launch(0, t_steps)
    maps0 = _prep_l0(src, emb, Wih0, Whh0, gih0, ghh0, h0, t_steps)
    res0 = run_bass_kernel_spmd(nc0, maps0, core_ids=list(range(NCORE)))
    y0 = np.zeros((t_steps, BK, 2 * H), np.float32)
    for c in range(NCORE):
        d, q = c // 4, c % 4
        y = res0.results[c]["yout"].astype(np.float32)
        if d == 1:
            y = y[::-1]
        y0[:, q * R:(q + 1) * R, d * H:(d + 1) * H] = y

    nc1 = _get_launch(1, t_steps)
    maps1 = _prep_l1(y0, Wih1, Whh1, gih1, ghh1, h0, t_steps)
    res1 = run_bass_kernel_spmd(nc1, maps1, core_ids=list(range(NCORE)))
    ybar = np.zeros((2, t_steps, BK, H), np.float32)
    for c in range(NCORE):
        d, q = c // 4, c % 4
        y = res1.results[c]["yout"].astype(np.float32)
        if d == 1:
            y = y[::-1]
        ybar[d, :, q * R:(q + 1) * R, :] = y

    out1 = 0.5 * (ybar[0] + ybar[1])
    out1 = np.ascontiguousarray(out1.transpose(1, 0, 2))
    lens = np.clip(lengths, 0, t_steps - 1)
    out2 = 0.5 * (ybar[1, 0, :, :] + ybar[0][lens, np.arange(BK), :])
    return out1, out2


def kernel(**inputs):
    return _run(inputs, T)
